# revision 26
# baseline (speedup 1.0000x reference)
"""VQVAE encoder forward on 8 Trainium2 NeuronCores (Bass/Tile).

Strategy (data-parallel over tokens, codebook replicated):
  - Each core processes 4096 tokens (= 8 full batch rows) per modality.
  - PE computes 2*x@e.T in fp32 (accurate to ~3e-8, enough to reproduce the
    reference's f32 argmin bit-for-bit given the exact rounding emulation).
  - VectorE assembles nd = fl(fl(-e_sq + -x_sq) + 2xe) = -fl(fl(e_sq+x_sq)-2xe),
    which mirrors the reference's distance rounding exactly; max/max_index give
    the argmin with first-index tie-breaking (matches jnp.argmin).
  - ScalarE computes z=sqrt(d), p=exp(-z) with a fused row-sum; PE accumulates
    per-batch-row sums of p/sum(p) (bf16) for the contrastive pH term.
  - Embedding rows are gathered by index with indirect DMA (the quantized
    output); per-token min distances are staged for the commitment losses.
  - Host does only O(N) / O(B*M) finishing: bincounts, modes, perplexity,
    the [64,400] contrastive Scode, and the cross-modal MSE reduction.
"""

import os
import sys

import numpy as np

for _p in ("/opt/trn_rl_repo", "/root/.axon_site/_ro/trn_rl_repo"):
    if os.path.isdir(_p) and _p not in sys.path:
        sys.path.append(_p)

import concourse.bass as bass
import concourse.mybir as mybir
import concourse.tile as tile
from concourse import bacc, bass_utils
from concourse.tile_rust import add_dep_helper

B, T, D, M = 64, 512, 256, 400
N = B * T
NCORES = 8
NSH = N // NCORES          # tokens per core (4096)
TILES = NSH // 128         # 32 token tiles per core
ROWS = B // NCORES         # batch rows per core (8)
TPR = T // 128             # tiles per batch row (4)

F32 = mybir.dt.float32
BF16 = mybir.dt.bfloat16
U32 = mybir.dt.uint32

COMMITMENT_COST = 0.25
EPSILON = 1e-5

LAST_EXEC_NS = None

_compiled = None


def _build():
    nc = bacc.Bacc("TRN2", target_bir_lowering=False, debug=False,
                   num_devices=NCORES)

    def din(name, shape, dt=F32):
        return nc.dram_tensor(name, shape, dt, kind="ExternalInput").ap()

    def dout(name, shape, dt=F32):
        return nc.dram_tensor(name, shape, dt, kind="ExternalOutput").ap()

    xT = {m: din(f"xT_{m}", [2, 128, NSH]) for m in "av"}      # x.T split into K-chunks
    nxsq = {m: din(f"nxsq_{m}", [128, TILES]) for m in "av"}   # -x_sq, [p, tile]
    e2pT = din("e2pT", [2, 128, M])                            # (2e).T K-chunks
    nesq = din("nesq", [M])                                    # -e_sq
    emb = din("emb", [M, D])                                   # raw codebook

    q_o = {m: dout(f"q_{m}", [NSH, D]) for m in "av"}
    idx_o = {m: dout(f"idx_{m}", [128, TILES, 8], U32) for m in "av"}
    md_o = {m: dout(f"md_{m}", [128, TILES, 8]) for m in "av"}  # top-8 of -d (host negates col 0)
    NGRP = 4
    ph_o = {m: dout(f"ph_{m}", [NGRP * ROWS, M]) for m in "av"}  # per-group partial pH sums

    prev_act = [None]

    def act_order(inst):
        raw = getattr(inst, "ins", inst)
        if prev_act[0] is not None:
            add_dep_helper(raw, prev_act[0], sync=False, reason="act table order")
        prev_act[0] = raw
        return inst

    with tile.TileContext(nc) as tc:
        with tc.tile_pool(name="big", bufs=1) as big, \
             tc.tile_pool(name="work", bufs=6) as work, \
             tc.tile_pool(name="small", bufs=32) as small, \
             tc.tile_pool(name="stage", bufs=1) as stage, \
             tc.tile_pool(name="zp", bufs=1) as zpool, \
             tc.tile_pool(name="ps", bufs=3, space="PSUM") as psp, \
             tc.tile_pool(name="psph", bufs=1, space="PSUM") as psph:

            # --- constants / whole-shard loads (once) ---
            xt_t = {}
            for m in "av":
                xt_t[m] = big.tile([128, 2, NSH], F32, tag=f"xt_{m}", name=f"xt_{m}")
                for q4 in range(8):
                    sl4 = bass.ts(q4, NSH // 8)
                    nc.sync.dma_start(
                        out=xt_t[m][:, :, sl4],
                        in_=xT[m].rearrange("c k t -> k c t")[:, :, sl4])
            e_t = big.tile([128, 2, M], F32, tag="e2p", name="e2p")
            nc.sync.dma_start(out=e_t, in_=e2pT.rearrange("c k m -> k c m"))
            nesq_t = big.tile([128, M], F32, tag="nesq", name="nesq_t")
            nesq_src = bass.AP(tensor=nesq.tensor, offset=nesq.offset,
                               ap=[[0, 128], [1, M]])
            nc.sync.dma_start(out=nesq_t, in_=nesq_src)
            nxsq_t = {}
            for m in "av":
                nxsq_t[m] = big.tile([128, TILES], F32, tag=f"nxsq_{m}", name=f"nxsq_{m}")
                nc.sync.dma_start(out=nxsq_t[m], in_=nxsq[m])

            idx_st = {m: stage.tile([128, TILES, 8], U32, tag=f"idxs_{m}", name=f"idxs_{m}") for m in "av"}
            md_st = {m: stage.tile([128, TILES, 8], F32, tag=f"mds_{m}", name=f"mds_{m}") for m in "av"}

            G = 8  # tiles per activation-batch group (amortizes ACT
                   # sqrt<->exp function-table reloads, 1.28us each)
            for g in range(0, TILES, G):
                zs = {}
                # phase A: matmul + exact-dist + argmin + all Sqrts
                for j in range(g, g + G):
                    for m in "av":
                        ps = psp.tile([128, M], F32, tag=f"xe_{m}", name=f"xe_{m}")
                        for c in range(2):
                            nc.tensor.matmul(ps, xt_t[m][:, c, bass.ts(j, 128)],
                                             e_t[:, c, :],
                                             start=(c == 0), stop=(c == 1))
                        nd = work.tile([128, M], F32, tag=f"nd_{m}", name=f"nd_{m}")
                        nc.vector.scalar_tensor_tensor(
                            nd, nesq_t, nxsq_t[m][:, j:j + 1], ps,
                            op0=mybir.AluOpType.add, op1=mybir.AluOpType.add)
                        z = zpool.tile([128, M], F32, tag=f"z_{m}{j % G}",
                                       name=f"z_{m}{j % G}")
                        act_order(nc.scalar.activation(
                            z, nd, mybir.ActivationFunctionType.Sqrt,
                            scale=-1.0))
                        zs[(j, m)] = z
                        mx8 = md_st[m][:, j, :]
                        ix8 = idx_st[m][:, j, :]
                        nc.vector.max(mx8, nd)
                        nc.vector.max_index(ix8, mx8, nd)
                # quantized rows: q = emb[idx]  (indirect DMA row gather)
                for j0 in range(g, g + G):
                    for m in "av":
                        qs = work.tile([128, D], F32, tag=f"q_{m}", name=f"q_{m}")
                        nc.gpsimd.indirect_dma_start(
                            out=qs, out_offset=None, in_=emb,
                            in_offset=bass.IndirectOffsetOnAxis(
                                ap=idx_st[m][:, j0, 0:1], axis=0))
                        nc.sync.dma_start(out=q_o[m][bass.ts(j0, 128), :], in_=qs)
                # phase B: all Exps + pH accumulation (per-group partials)
                ph_ps = {m: psph.tile([ROWS, M], F32, tag=f"php_{m}",
                                      name=f"php_{m}") for m in "av"}
                for j in range(g, g + G):
                    row = j // TPR
                    for m in "av":
                        pb = work.tile([128, M], BF16, tag=f"pb_{m}", name=f"pb_{m}")
                        se = small.tile([128, 1], F32, tag=f"se_{m}", name=f"se_{m}")
                        act_order(nc.scalar.activation(
                            pb, zs[(j, m)], mybir.ActivationFunctionType.Exp,
                            scale=-1.0, accum_out=se))
                        lhs = small.tile([128, ROWS], BF16, tag=f"lhs_{m}", name=f"lhs_{m}")
                        nc.gpsimd.memset(lhs, 0.0)
                        with nc.allow_low_precision(
                                reason="pH weights tolerate bf16 (1e-3 budget)"):
                            nc.vector.reciprocal(lhs[:, row:row + 1], se)
                        nc.tensor.matmul(ph_ps[m], lhs, pb,
                                         start=(j == g), stop=(j == g + G - 1))

                # stage out this group's pH partial
                gi = g // G
                for m in "av":
                    phs = stage.tile([ROWS, M], F32, tag=f"phs_{m}{gi}",
                                     name=f"phs_{m}{gi}")
                    nc.scalar.copy(phs, ph_ps[m])
                    nc.sync.dma_start(
                        out=ph_o[m][gi * ROWS:(gi + 1) * ROWS, :], in_=phs)

            # --- epilogue: stages out ---
            for m in "av":
                nc.sync.dma_start(out=idx_o[m], in_=idx_st[m])
                nc.sync.dma_start(out=md_o[m], in_=md_st[m])

    nc.compile()
    return nc


def _xla_rowsum_sq(x):
    """Bit-exact emulation of XLA-CPU f32 jnp.sum(x*x, axis=1) for D=256:
    reduce-window(32) sequential in-window, then sequential over 8 windows."""
    x2 = (x * x).astype(np.float32)
    b = x2.reshape(x2.shape[0], 8, 32)
    s = b[:, :, 0].copy()
    for t in range(1, 32):
        s = (s + b[:, :, t]).astype(np.float32)
    out = s[:, 0].copy()
    for k in range(1, 8):
        out = (out + s[:, k]).astype(np.float32)
    return out


def kernel(audio_semantic, video_semantic, embedding):
    global _compiled, LAST_EXEC_NS
    audio_semantic = np.asarray(audio_semantic, dtype=np.float32)
    video_semantic = np.asarray(video_semantic, dtype=np.float32)
    embedding = np.asarray(embedding, dtype=np.float32)

    if _compiled is None:
        _compiled = _build()
    nc = _compiled

    a_flat = audio_semantic.reshape(N, D)
    v_flat = video_semantic.reshape(N, D)
    e_sq = _xla_rowsum_sq(embedding)
    x_sq = {"a": _xla_rowsum_sq(a_flat), "v": _xla_rowsum_sq(v_flat)}
    xT_full = {"a": a_flat.T, "v": v_flat.T}
    e2pT = np.ascontiguousarray((2.0 * embedding).T).reshape(2, 128, M)
    nesq = np.ascontiguousarray(-e_sq)

    in_maps = []
    for c in range(NCORES):
        sl = slice(c * NSH, (c + 1) * NSH)
        im = {"e2pT": e2pT, "nesq": nesq, "emb": embedding}
        for m, xt in xT_full.items():
            im[f"xT_{m}"] = np.ascontiguousarray(xt[:, sl]).reshape(2, 128, NSH)
            im[f"nxsq_{m}"] = np.ascontiguousarray(
                (-x_sq[m][sl]).reshape(TILES, 128).T)
        in_maps.append(im)

    import time
    t0 = time.perf_counter()
    res = bass_utils.run_bass_kernel_spmd(
        nc, in_maps, core_ids=list(range(NCORES)))
    t1 = time.perf_counter()
    LAST_EXEC_NS = res.exec_time_ns
    if LAST_EXEC_NS is None:
        LAST_EXEC_NS = int((t1 - t0) * 1e9)

    # --- host finishing ---
    qf = {}
    idx = {}
    mind = {}
    pH = {}
    for m in "av":
        qf[m] = np.concatenate([res.results[c][f"q_{m}"] for c in range(NCORES)], 0)
        idx[m] = np.concatenate(
            [res.results[c][f"idx_{m}"][:, :, 0].T.reshape(NSH)
             for c in range(NCORES)])
        mind[m] = np.concatenate(
            [-res.results[c][f"md_{m}"][:, :, 0].T.reshape(NSH)
             for c in range(NCORES)])
        pH[m] = np.concatenate(
            [res.results[c][f"ph_{m}"].reshape(-1, ROWS, M).sum(0)
             for c in range(NCORES)], 0) / np.float32(T)

    # straight-through estimator: x + (q - x) in f32, matching the
    # reference's elementwise rounding bit-for-bit.
    a_q = (a_flat + (qf["a"] - a_flat)).reshape(B, T, D)
    v_q = (v_flat + (qf["v"] - v_flat)).reshape(B, T, D)

    # commitment losses: same-modality term == mean of min distances;
    # cross terms from the gathered quantized tensors (f64 accumulation).
    nd_ = float(N) * float(D)
    a_e = mind["a"].astype(np.float64).sum() / nd_
    v_e = mind["v"].astype(np.float64).sum() / nd_
    av_e = np.mean((a_flat.astype(np.float64) - qf["v"].astype(np.float64)) ** 2)
    va_e = np.mean((v_flat.astype(np.float64) - qf["a"].astype(np.float64)) ** 2)
    a_loss = np.float32(COMMITMENT_COST * 2.0 * a_e + COMMITMENT_COST * av_e)
    v_loss = np.float32(COMMITMENT_COST * 2.0 * v_e + COMMITMENT_COST * va_e)

    # perplexity
    def perplexity(ix):
        cnt = np.bincount(ix, minlength=M).astype(np.float64) / N
        return np.float32(np.exp(-np.sum(cnt * np.log(cnt + 1e-10))))

    a_perplexity = perplexity(idx["a"])
    v_perplexity = perplexity(idx["v"])

    # row modes + agreement
    def modes(ix):
        r = ix.reshape(B, T)
        cnt = np.zeros((B, M), np.int64)
        np.add.at(cnt, (np.repeat(np.arange(B), T), r.ravel()), 1)
        return cnt.argmax(1)

    equal_num = np.int32((modes(idx["a"]) == modes(idx["v"])).sum())

    # contrastive loss from pH
    a_pH = pH["a"].astype(np.float64)
    v_pH = pH["v"].astype(np.float64)
    Scode = a_pH @ np.log(v_pH.T + 1e-10) + v_pH @ np.log(a_pH.T + 1e-10)
    MaxScode = np.max(-Scode)
    EScode = np.exp(Scode + MaxScode)
    Lcmcm = -np.mean(np.log(np.diagonal(EScode) / (EScode.sum(1) + EPSILON)))
    cmcm_loss = np.float32(0.5 * Lcmcm)

    return (a_q, v_q, a_loss, v_loss,
            a_perplexity, v_perplexity, cmcm_loss, equal_num)


# revision 27
# speedup vs baseline: 1.0331x; 1.0331x over previous
"""VQVAE encoder forward on 8 Trainium2 NeuronCores (Bass/Tile).

Strategy (data-parallel over tokens, codebook replicated):
  - Each core processes 4096 tokens (= 8 full batch rows) per modality.
  - PE computes 2*x@e.T in fp32 (accurate to ~3e-8, enough to reproduce the
    reference's f32 argmin bit-for-bit given the exact rounding emulation).
  - VectorE assembles nd = fl(fl(-e_sq + -x_sq) + 2xe) = -fl(fl(e_sq+x_sq)-2xe),
    which mirrors the reference's distance rounding exactly; max/max_index give
    the argmin with first-index tie-breaking (matches jnp.argmin).
  - ScalarE computes z=sqrt(d), p=exp(-z) with a fused row-sum; PE accumulates
    per-batch-row sums of p/sum(p) (bf16) for the contrastive pH term.
  - Embedding rows are gathered by index with indirect DMA (the quantized
    output); per-token min distances are staged for the commitment losses.
  - Host does only O(N) / O(B*M) finishing: bincounts, modes, perplexity,
    the [64,400] contrastive Scode, and the cross-modal MSE reduction.
"""

import os
import sys

import numpy as np

for _p in ("/opt/trn_rl_repo", "/root/.axon_site/_ro/trn_rl_repo"):
    if os.path.isdir(_p) and _p not in sys.path:
        sys.path.append(_p)

import concourse.bass as bass
import concourse.mybir as mybir
import concourse.tile as tile
from concourse import bacc, bass_utils
from concourse.tile_rust import add_dep_helper

B, T, D, M = 64, 512, 256, 400
N = B * T
NCORES = 8
NSH = N // NCORES          # tokens per core (4096)
TILES = NSH // 128         # 32 token tiles per core
ROWS = B // NCORES         # batch rows per core (8)
TPR = T // 128             # tiles per batch row (4)

F32 = mybir.dt.float32
BF16 = mybir.dt.bfloat16
U32 = mybir.dt.uint32

COMMITMENT_COST = 0.25
EPSILON = 1e-5

LAST_EXEC_NS = None

_compiled = None


def _build():
    nc = bacc.Bacc("TRN2", target_bir_lowering=False, debug=False,
                   num_devices=NCORES)

    def din(name, shape, dt=F32):
        return nc.dram_tensor(name, shape, dt, kind="ExternalInput").ap()

    def dout(name, shape, dt=F32):
        return nc.dram_tensor(name, shape, dt, kind="ExternalOutput").ap()

    xT = {m: din(f"xT_{m}", [2, 128, NSH]) for m in "av"}      # x.T split into K-chunks
    nxsq = {m: din(f"nxsq_{m}", [128, TILES]) for m in "av"}   # -x_sq, [p, tile]
    e2pT = din("e2pT", [2, 128, M])                            # (2e).T K-chunks
    nesq = din("nesq", [M])                                    # -e_sq
    emb = din("emb", [M, D])                                   # raw codebook

    q_o = {m: dout(f"q_{m}", [NSH, D]) for m in "av"}
    idx_o = {m: dout(f"idx_{m}", [128, TILES, 8], U32) for m in "av"}
    md_o = {m: dout(f"md_{m}", [128, TILES, 8]) for m in "av"}  # top-8 of -d (host negates col 0)
    NGRP = 4
    ph_o = {m: dout(f"ph_{m}", [NGRP * ROWS, M]) for m in "av"}  # per-group partial pH sums

    prev_act = [None]

    def act_order(inst):
        raw = getattr(inst, "ins", inst)
        if prev_act[0] is not None:
            add_dep_helper(raw, prev_act[0], sync=False, reason="act table order")
        prev_act[0] = raw
        return inst

    with tile.TileContext(nc) as tc:
        with tc.tile_pool(name="big", bufs=1) as big, \
             tc.tile_pool(name="work", bufs=6) as work, \
             tc.tile_pool(name="small", bufs=32) as small, \
             tc.tile_pool(name="stage", bufs=1) as stage, \
             tc.tile_pool(name="zp", bufs=1) as zpool, \
             tc.tile_pool(name="ps", bufs=3, space="PSUM") as psp, \
             tc.tile_pool(name="psph", bufs=1, space="PSUM") as psph:

            # --- constants / whole-shard loads (once) ---
            xt_t = {}
            for m in "av":
                xt_t[m] = big.tile([128, 2, NSH], F32, tag=f"xt_{m}", name=f"xt_{m}")
                for q4 in range(8):
                    sl4 = bass.ts(q4, NSH // 8)
                    nc.sync.dma_start(
                        out=xt_t[m][:, :, sl4],
                        in_=xT[m].rearrange("c k t -> k c t")[:, :, sl4])
            e_t = big.tile([128, 2, M], F32, tag="e2p", name="e2p")
            nc.sync.dma_start(out=e_t, in_=e2pT.rearrange("c k m -> k c m"))
            nesq_t = big.tile([128, M], F32, tag="nesq", name="nesq_t")
            nesq_src = bass.AP(tensor=nesq.tensor, offset=nesq.offset,
                               ap=[[0, 128], [1, M]])
            nc.sync.dma_start(out=nesq_t, in_=nesq_src)
            nxsq_t = {}
            for m in "av":
                nxsq_t[m] = big.tile([128, TILES], F32, tag=f"nxsq_{m}", name=f"nxsq_{m}")
                nc.sync.dma_start(out=nxsq_t[m], in_=nxsq[m])

            idx_st = {m: stage.tile([128, TILES, 8], U32, tag=f"idxs_{m}", name=f"idxs_{m}") for m in "av"}
            md_st = {m: stage.tile([128, TILES, 8], F32, tag=f"mds_{m}", name=f"mds_{m}") for m in "av"}

            G = 8  # tiles per activation-batch group (amortizes ACT
                   # sqrt<->exp function-table reloads, 1.28us each)
            for g in range(0, TILES, G):
                zs = {}
                # phase A: matmul + exact-dist + argmin + all Sqrts.
                # Both modalities of a tile share one [128,2,512] PSUM pair
                # (one bank each) so the exact-dist add and the sqrt run as
                # single fused [128,2,400] passes, halving per-op overheads.
                for j in range(g, g + G):
                    ps = psp.tile([128, 2, 512], F32, tag="xe", name="xe")
                    sneg = work.tile([128, 2, M], F32, tag="sneg", name="sneg")
                    for mi, m in enumerate("av"):
                        for c in range(2):
                            nc.tensor.matmul(ps[:, mi, 0:M],
                                             xt_t[m][:, c, bass.ts(j, 128)],
                                             e_t[:, c, :],
                                             start=(c == 0), stop=(c == 1))
                        # sneg = fl(-e_sq + -x_sq): same rounding as the
                        # reference's fl(e_sq + x_sq), negated (gpsimd IEEE f32)
                        nc.gpsimd.tensor_scalar_add(sneg[:, mi, :], nesq_t,
                                                    nxsq_t[m][:, j:j + 1])
                    nd = work.tile([128, 2, M], F32, tag="nd", name="nd")
                    nc.vector.tensor_tensor(nd, sneg, ps[:, :, 0:M],
                                            op=mybir.AluOpType.add)
                    z = zpool.tile([128, 2, M], F32, tag=f"z{j % G}",
                                   name=f"z{j % G}")
                    act_order(nc.scalar.activation(
                        z, nd, mybir.ActivationFunctionType.Sqrt,
                        scale=-1.0))
                    for mi, m in enumerate("av"):
                        zs[(j, m)] = z[:, mi, :]
                        mx8 = md_st[m][:, j, :]
                        ix8 = idx_st[m][:, j, :]
                        nc.vector.max(mx8, nd[:, mi, :])
                        nc.vector.max_index(ix8, mx8, nd[:, mi, :])
                # quantized rows: q = emb[idx]  (indirect DMA row gather)
                for j0 in range(g, g + G):
                    for m in "av":
                        qs = work.tile([128, D], F32, tag=f"q_{m}", name=f"q_{m}")
                        nc.gpsimd.indirect_dma_start(
                            out=qs, out_offset=None, in_=emb,
                            in_offset=bass.IndirectOffsetOnAxis(
                                ap=idx_st[m][:, j0, 0:1], axis=0))
                        nc.sync.dma_start(out=q_o[m][bass.ts(j0, 128), :], in_=qs)
                # phase B: all Exps + pH accumulation (per-group partials)
                ph_ps = {m: psph.tile([ROWS, M], F32, tag=f"php_{m}",
                                      name=f"php_{m}") for m in "av"}
                for j in range(g, g + G):
                    row = j // TPR
                    for m in "av":
                        pb = work.tile([128, M], BF16, tag=f"pb_{m}", name=f"pb_{m}")
                        se = small.tile([128, 1], F32, tag=f"se_{m}", name=f"se_{m}")
                        act_order(nc.scalar.activation(
                            pb, zs[(j, m)], mybir.ActivationFunctionType.Exp,
                            scale=-1.0, accum_out=se))
                        lhs = small.tile([128, ROWS], BF16, tag=f"lhs_{m}", name=f"lhs_{m}")
                        nc.gpsimd.memset(lhs, 0.0)
                        with nc.allow_low_precision(
                                reason="pH weights tolerate bf16 (1e-3 budget)"):
                            nc.vector.reciprocal(lhs[:, row:row + 1], se)
                        nc.tensor.matmul(ph_ps[m], lhs, pb,
                                         start=(j == g), stop=(j == g + G - 1))

                # stage out this group's pH partial
                gi = g // G
                for m in "av":
                    phs = stage.tile([ROWS, M], F32, tag=f"phs_{m}{gi}",
                                     name=f"phs_{m}{gi}")
                    nc.scalar.copy(phs, ph_ps[m])
                    nc.sync.dma_start(
                        out=ph_o[m][gi * ROWS:(gi + 1) * ROWS, :], in_=phs)

            # --- epilogue: stages out ---
            for m in "av":
                nc.sync.dma_start(out=idx_o[m], in_=idx_st[m])
                nc.sync.dma_start(out=md_o[m], in_=md_st[m])

    nc.compile()
    return nc


def _xla_rowsum_sq(x):
    """Bit-exact emulation of XLA-CPU f32 jnp.sum(x*x, axis=1) for D=256:
    reduce-window(32) sequential in-window, then sequential over 8 windows."""
    x2 = (x * x).astype(np.float32)
    b = x2.reshape(x2.shape[0], 8, 32)
    s = b[:, :, 0].copy()
    for t in range(1, 32):
        s = (s + b[:, :, t]).astype(np.float32)
    out = s[:, 0].copy()
    for k in range(1, 8):
        out = (out + s[:, k]).astype(np.float32)
    return out


def kernel(audio_semantic, video_semantic, embedding):
    global _compiled, LAST_EXEC_NS
    audio_semantic = np.asarray(audio_semantic, dtype=np.float32)
    video_semantic = np.asarray(video_semantic, dtype=np.float32)
    embedding = np.asarray(embedding, dtype=np.float32)

    if _compiled is None:
        _compiled = _build()
    nc = _compiled

    a_flat = audio_semantic.reshape(N, D)
    v_flat = video_semantic.reshape(N, D)
    e_sq = _xla_rowsum_sq(embedding)
    x_sq = {"a": _xla_rowsum_sq(a_flat), "v": _xla_rowsum_sq(v_flat)}
    xT_full = {"a": a_flat.T, "v": v_flat.T}
    e2pT = np.ascontiguousarray((2.0 * embedding).T).reshape(2, 128, M)
    nesq = np.ascontiguousarray(-e_sq)

    in_maps = []
    for c in range(NCORES):
        sl = slice(c * NSH, (c + 1) * NSH)
        im = {"e2pT": e2pT, "nesq": nesq, "emb": embedding}
        for m, xt in xT_full.items():
            im[f"xT_{m}"] = np.ascontiguousarray(xt[:, sl]).reshape(2, 128, NSH)
            im[f"nxsq_{m}"] = np.ascontiguousarray(
                (-x_sq[m][sl]).reshape(TILES, 128).T)
        in_maps.append(im)

    import time
    t0 = time.perf_counter()
    res = bass_utils.run_bass_kernel_spmd(
        nc, in_maps, core_ids=list(range(NCORES)))
    t1 = time.perf_counter()
    LAST_EXEC_NS = res.exec_time_ns
    if LAST_EXEC_NS is None:
        LAST_EXEC_NS = int((t1 - t0) * 1e9)

    # --- host finishing ---
    qf = {}
    idx = {}
    mind = {}
    pH = {}
    for m in "av":
        qf[m] = np.concatenate([res.results[c][f"q_{m}"] for c in range(NCORES)], 0)
        idx[m] = np.concatenate(
            [res.results[c][f"idx_{m}"][:, :, 0].T.reshape(NSH)
             for c in range(NCORES)])
        mind[m] = np.concatenate(
            [-res.results[c][f"md_{m}"][:, :, 0].T.reshape(NSH)
             for c in range(NCORES)])
        pH[m] = np.concatenate(
            [res.results[c][f"ph_{m}"].reshape(-1, ROWS, M).sum(0)
             for c in range(NCORES)], 0) / np.float32(T)

    # straight-through estimator: x + (q - x) in f32, matching the
    # reference's elementwise rounding bit-for-bit.
    a_q = (a_flat + (qf["a"] - a_flat)).reshape(B, T, D)
    v_q = (v_flat + (qf["v"] - v_flat)).reshape(B, T, D)

    # commitment losses: same-modality term == mean of min distances;
    # cross terms from the gathered quantized tensors (f64 accumulation).
    nd_ = float(N) * float(D)
    a_e = mind["a"].astype(np.float64).sum() / nd_
    v_e = mind["v"].astype(np.float64).sum() / nd_
    av_e = np.mean((a_flat.astype(np.float64) - qf["v"].astype(np.float64)) ** 2)
    va_e = np.mean((v_flat.astype(np.float64) - qf["a"].astype(np.float64)) ** 2)
    a_loss = np.float32(COMMITMENT_COST * 2.0 * a_e + COMMITMENT_COST * av_e)
    v_loss = np.float32(COMMITMENT_COST * 2.0 * v_e + COMMITMENT_COST * va_e)

    # perplexity
    def perplexity(ix):
        cnt = np.bincount(ix, minlength=M).astype(np.float64) / N
        return np.float32(np.exp(-np.sum(cnt * np.log(cnt + 1e-10))))

    a_perplexity = perplexity(idx["a"])
    v_perplexity = perplexity(idx["v"])

    # row modes + agreement
    def modes(ix):
        r = ix.reshape(B, T)
        cnt = np.zeros((B, M), np.int64)
        np.add.at(cnt, (np.repeat(np.arange(B), T), r.ravel()), 1)
        return cnt.argmax(1)

    equal_num = np.int32((modes(idx["a"]) == modes(idx["v"])).sum())

    # contrastive loss from pH
    a_pH = pH["a"].astype(np.float64)
    v_pH = pH["v"].astype(np.float64)
    Scode = a_pH @ np.log(v_pH.T + 1e-10) + v_pH @ np.log(a_pH.T + 1e-10)
    MaxScode = np.max(-Scode)
    EScode = np.exp(Scode + MaxScode)
    Lcmcm = -np.mean(np.log(np.diagonal(EScode) / (EScode.sum(1) + EPSILON)))
    cmcm_loss = np.float32(0.5 * Lcmcm)

    return (a_q, v_q, a_loss, v_loss,
            a_perplexity, v_perplexity, cmcm_loss, equal_num)


# revision 28
# speedup vs baseline: 1.0336x; 1.0005x over previous
"""VQVAE encoder forward on 8 Trainium2 NeuronCores (Bass/Tile).

Strategy (data-parallel over tokens, codebook replicated):
  - Each core processes 4096 tokens (= 8 full batch rows) per modality.
  - PE computes 2*x@e.T in fp32 (accurate to ~3e-8, enough to reproduce the
    reference's f32 argmin bit-for-bit given the exact rounding emulation).
  - VectorE assembles nd = fl(fl(-e_sq + -x_sq) + 2xe) = -fl(fl(e_sq+x_sq)-2xe),
    which mirrors the reference's distance rounding exactly; max/max_index give
    the argmin with first-index tie-breaking (matches jnp.argmin).
  - ScalarE computes z=sqrt(d), p=exp(-z) with a fused row-sum; PE accumulates
    per-batch-row sums of p/sum(p) (bf16) for the contrastive pH term.
  - Embedding rows are gathered by index with indirect DMA (the quantized
    output); per-token min distances are staged for the commitment losses.
  - Host does only O(N) / O(B*M) finishing: bincounts, modes, perplexity,
    the [64,400] contrastive Scode, and the cross-modal MSE reduction.
"""

import os
import sys

import numpy as np

for _p in ("/opt/trn_rl_repo", "/root/.axon_site/_ro/trn_rl_repo"):
    if os.path.isdir(_p) and _p not in sys.path:
        sys.path.append(_p)

import concourse.bass as bass
import concourse.mybir as mybir
import concourse.tile as tile
from concourse import bacc, bass_utils
from concourse.tile_rust import add_dep_helper

B, T, D, M = 64, 512, 256, 400
N = B * T
NCORES = 8
NSH = N // NCORES          # tokens per core (4096)
TILES = NSH // 128         # 32 token tiles per core
ROWS = B // NCORES         # batch rows per core (8)
TPR = T // 128             # tiles per batch row (4)

F32 = mybir.dt.float32
BF16 = mybir.dt.bfloat16
U32 = mybir.dt.uint32

COMMITMENT_COST = 0.25
EPSILON = 1e-5

LAST_EXEC_NS = None

_compiled = None


def _build():
    nc = bacc.Bacc("TRN2", target_bir_lowering=False, debug=False,
                   num_devices=NCORES)

    def din(name, shape, dt=F32):
        return nc.dram_tensor(name, shape, dt, kind="ExternalInput").ap()

    def dout(name, shape, dt=F32):
        return nc.dram_tensor(name, shape, dt, kind="ExternalOutput").ap()

    xT = {m: din(f"xT_{m}", [2, 128, NSH]) for m in "av"}      # x.T split into K-chunks
    nxsq = {m: din(f"nxsq_{m}", [128, TILES]) for m in "av"}   # -x_sq, [p, tile]
    e2pT = din("e2pT", [2, 128, M])                            # (2e).T K-chunks
    nesq = din("nesq", [M])                                    # -e_sq
    emb = din("emb", [M, D])                                   # raw codebook

    q_o = {m: dout(f"q_{m}", [NSH, D]) for m in "av"}
    idx_o = {m: dout(f"idx_{m}", [128, TILES, 8], U32) for m in "av"}
    md_o = {m: dout(f"md_{m}", [128, TILES, 8]) for m in "av"}  # top-8 of -d (host negates col 0)
    NGRP = 4
    ph_o = {m: dout(f"ph_{m}", [NGRP * ROWS, M]) for m in "av"}  # per-group partial pH sums

    prev_act = [None]

    def act_order(inst):
        raw = getattr(inst, "ins", inst)
        if prev_act[0] is not None:
            add_dep_helper(raw, prev_act[0], sync=False, reason="act table order")
        prev_act[0] = raw
        return inst

    with tile.TileContext(nc) as tc:
        with tc.tile_pool(name="big", bufs=1) as big, \
             tc.tile_pool(name="work", bufs=6) as work, \
             tc.tile_pool(name="small", bufs=32) as small, \
             tc.tile_pool(name="stage", bufs=1) as stage, \
             tc.tile_pool(name="zp", bufs=2) as zpool, \
             tc.tile_pool(name="ps", bufs=3, space="PSUM") as psp, \
             tc.tile_pool(name="psph", bufs=1, space="PSUM") as psph:

            # --- constants / whole-shard loads (once) ---
            xt_t = {}
            for m in "av":
                xt_t[m] = big.tile([128, 2, NSH], F32, tag=f"xt_{m}", name=f"xt_{m}")
                for q4 in range(8):
                    sl4 = bass.ts(q4, NSH // 8)
                    nc.sync.dma_start(
                        out=xt_t[m][:, :, sl4],
                        in_=xT[m].rearrange("c k t -> k c t")[:, :, sl4])
            e_t = big.tile([128, 2, M], F32, tag="e2p", name="e2p")
            nc.sync.dma_start(out=e_t, in_=e2pT.rearrange("c k m -> k c m"))
            nesq_t = big.tile([128, M], F32, tag="nesq", name="nesq_t")
            nesq_src = bass.AP(tensor=nesq.tensor, offset=nesq.offset,
                               ap=[[0, 128], [1, M]])
            nc.sync.dma_start(out=nesq_t, in_=nesq_src)
            nxsq_t = {}
            for m in "av":
                nxsq_t[m] = big.tile([128, TILES], F32, tag=f"nxsq_{m}", name=f"nxsq_{m}")
                nc.sync.dma_start(out=nxsq_t[m], in_=nxsq[m])

            idx_st = {m: stage.tile([128, TILES, 8], U32, tag=f"idxs_{m}", name=f"idxs_{m}") for m in "av"}
            md_st = {m: stage.tile([128, TILES, 8], F32, tag=f"mds_{m}", name=f"mds_{m}") for m in "av"}

            G = 8  # tiles per activation-batch group (amortizes ACT
                   # sqrt<->exp function-table reloads, 1.28us each)
            for g in range(0, TILES, G):
                zs = {}
                # phase A: matmul + exact-dist + argmin + all Sqrts.
                # Both modalities of a tile share one [128,2,512] PSUM pair
                # (one bank each) so the exact-dist add and the sqrt run as
                # single fused [128,2,400] passes, halving per-op overheads.
                for j in range(g, g + G):
                    ps = psp.tile([128, 2, 512], F32, tag="xe", name="xe")
                    sneg = work.tile([128, 2, M], F32, tag="sneg", name="sneg")
                    for mi, m in enumerate("av"):
                        for c in range(2):
                            nc.tensor.matmul(ps[:, mi, 0:M],
                                             xt_t[m][:, c, bass.ts(j, 128)],
                                             e_t[:, c, :],
                                             start=(c == 0), stop=(c == 1))
                        # sneg = fl(-e_sq + -x_sq): same rounding as the
                        # reference's fl(e_sq + x_sq), negated (gpsimd IEEE f32)
                        nc.gpsimd.tensor_scalar_add(sneg[:, mi, :], nesq_t,
                                                    nxsq_t[m][:, j:j + 1])
                    nd = work.tile([128, 2, M], F32, tag="nd", name="nd")
                    nc.vector.tensor_tensor(nd, sneg, ps[:, :, 0:M],
                                            op=mybir.AluOpType.add)
                    z = zpool.tile([128, 2, M], F32, tag=f"z{j % G}",
                                   name=f"z{j % G}")
                    act_order(nc.scalar.activation(
                        z, nd, mybir.ActivationFunctionType.Sqrt,
                        scale=-1.0))
                    for mi, m in enumerate("av"):
                        zs[(j, m)] = z[:, mi, :]
                        mx8 = md_st[m][:, j, :]
                        ix8 = idx_st[m][:, j, :]
                        nc.vector.max(mx8, nd[:, mi, :])
                        nc.vector.max_index(ix8, mx8, nd[:, mi, :])
                # quantized rows: q = emb[idx]  (indirect DMA row gather)
                for j0 in range(g, g + G):
                    for m in "av":
                        qs = work.tile([128, D], F32, tag=f"q_{m}", name=f"q_{m}")
                        nc.gpsimd.indirect_dma_start(
                            out=qs, out_offset=None, in_=emb,
                            in_offset=bass.IndirectOffsetOnAxis(
                                ap=idx_st[m][:, j0, 0:1], axis=0))
                        nc.sync.dma_start(out=q_o[m][bass.ts(j0, 128), :], in_=qs)
                # phase B: all Exps + pH accumulation (per-group partials)
                ph_ps = {m: psph.tile([ROWS, M], F32, tag=f"php_{m}",
                                      name=f"php_{m}") for m in "av"}
                for j in range(g, g + G):
                    row = j // TPR
                    for m in "av":
                        pb = work.tile([128, M], BF16, tag=f"pb_{m}", name=f"pb_{m}")
                        se = small.tile([128, 1], F32, tag=f"se_{m}", name=f"se_{m}")
                        act_order(nc.scalar.activation(
                            pb, zs[(j, m)], mybir.ActivationFunctionType.Exp,
                            scale=-1.0, accum_out=se))
                        lhs = small.tile([128, ROWS], BF16, tag=f"lhs_{m}", name=f"lhs_{m}")
                        nc.gpsimd.memset(lhs, 0.0)
                        with nc.allow_low_precision(
                                reason="pH weights tolerate bf16 (1e-3 budget)"):
                            nc.vector.reciprocal(lhs[:, row:row + 1], se)
                        nc.tensor.matmul(ph_ps[m], lhs, pb,
                                         start=(j == g), stop=(j == g + G - 1))

                # stage out this group's pH partial
                gi = g // G
                for m in "av":
                    phs = stage.tile([ROWS, M], F32, tag=f"phs_{m}{gi}",
                                     name=f"phs_{m}{gi}")
                    nc.scalar.copy(phs, ph_ps[m])
                    nc.sync.dma_start(
                        out=ph_o[m][gi * ROWS:(gi + 1) * ROWS, :], in_=phs)

            # --- epilogue: stages out ---
            for m in "av":
                nc.sync.dma_start(out=idx_o[m], in_=idx_st[m])
                nc.sync.dma_start(out=md_o[m], in_=md_st[m])

    nc.compile()
    return nc


def _xla_rowsum_sq(x):
    """Bit-exact emulation of XLA-CPU f32 jnp.sum(x*x, axis=1) for D=256:
    reduce-window(32) sequential in-window, then sequential over 8 windows."""
    x2 = (x * x).astype(np.float32)
    b = x2.reshape(x2.shape[0], 8, 32)
    s = b[:, :, 0].copy()
    for t in range(1, 32):
        s = (s + b[:, :, t]).astype(np.float32)
    out = s[:, 0].copy()
    for k in range(1, 8):
        out = (out + s[:, k]).astype(np.float32)
    return out


def kernel(audio_semantic, video_semantic, embedding):
    global _compiled, LAST_EXEC_NS
    audio_semantic = np.asarray(audio_semantic, dtype=np.float32)
    video_semantic = np.asarray(video_semantic, dtype=np.float32)
    embedding = np.asarray(embedding, dtype=np.float32)

    if _compiled is None:
        _compiled = _build()
    nc = _compiled

    a_flat = audio_semantic.reshape(N, D)
    v_flat = video_semantic.reshape(N, D)
    e_sq = _xla_rowsum_sq(embedding)
    x_sq = {"a": _xla_rowsum_sq(a_flat), "v": _xla_rowsum_sq(v_flat)}
    xT_full = {"a": a_flat.T, "v": v_flat.T}
    e2pT = np.ascontiguousarray((2.0 * embedding).T).reshape(2, 128, M)
    nesq = np.ascontiguousarray(-e_sq)

    in_maps = []
    for c in range(NCORES):
        sl = slice(c * NSH, (c + 1) * NSH)
        im = {"e2pT": e2pT, "nesq": nesq, "emb": embedding}
        for m, xt in xT_full.items():
            im[f"xT_{m}"] = np.ascontiguousarray(xt[:, sl]).reshape(2, 128, NSH)
            im[f"nxsq_{m}"] = np.ascontiguousarray(
                (-x_sq[m][sl]).reshape(TILES, 128).T)
        in_maps.append(im)

    import time
    t0 = time.perf_counter()
    res = bass_utils.run_bass_kernel_spmd(
        nc, in_maps, core_ids=list(range(NCORES)))
    t1 = time.perf_counter()
    LAST_EXEC_NS = res.exec_time_ns
    if LAST_EXEC_NS is None:
        LAST_EXEC_NS = int((t1 - t0) * 1e9)

    # --- host finishing ---
    qf = {}
    idx = {}
    mind = {}
    pH = {}
    for m in "av":
        qf[m] = np.concatenate([res.results[c][f"q_{m}"] for c in range(NCORES)], 0)
        idx[m] = np.concatenate(
            [res.results[c][f"idx_{m}"][:, :, 0].T.reshape(NSH)
             for c in range(NCORES)])
        mind[m] = np.concatenate(
            [-res.results[c][f"md_{m}"][:, :, 0].T.reshape(NSH)
             for c in range(NCORES)])
        pH[m] = np.concatenate(
            [res.results[c][f"ph_{m}"].reshape(-1, ROWS, M).sum(0)
             for c in range(NCORES)], 0) / np.float32(T)

    # straight-through estimator: x + (q - x) in f32, matching the
    # reference's elementwise rounding bit-for-bit.
    a_q = (a_flat + (qf["a"] - a_flat)).reshape(B, T, D)
    v_q = (v_flat + (qf["v"] - v_flat)).reshape(B, T, D)

    # commitment losses: same-modality term == mean of min distances;
    # cross terms from the gathered quantized tensors (f64 accumulation).
    nd_ = float(N) * float(D)
    a_e = mind["a"].astype(np.float64).sum() / nd_
    v_e = mind["v"].astype(np.float64).sum() / nd_
    av_e = np.mean((a_flat.astype(np.float64) - qf["v"].astype(np.float64)) ** 2)
    va_e = np.mean((v_flat.astype(np.float64) - qf["a"].astype(np.float64)) ** 2)
    a_loss = np.float32(COMMITMENT_COST * 2.0 * a_e + COMMITMENT_COST * av_e)
    v_loss = np.float32(COMMITMENT_COST * 2.0 * v_e + COMMITMENT_COST * va_e)

    # perplexity
    def perplexity(ix):
        cnt = np.bincount(ix, minlength=M).astype(np.float64) / N
        return np.float32(np.exp(-np.sum(cnt * np.log(cnt + 1e-10))))

    a_perplexity = perplexity(idx["a"])
    v_perplexity = perplexity(idx["v"])

    # row modes + agreement
    def modes(ix):
        r = ix.reshape(B, T)
        cnt = np.zeros((B, M), np.int64)
        np.add.at(cnt, (np.repeat(np.arange(B), T), r.ravel()), 1)
        return cnt.argmax(1)

    equal_num = np.int32((modes(idx["a"]) == modes(idx["v"])).sum())

    # contrastive loss from pH
    a_pH = pH["a"].astype(np.float64)
    v_pH = pH["v"].astype(np.float64)
    Scode = a_pH @ np.log(v_pH.T + 1e-10) + v_pH @ np.log(a_pH.T + 1e-10)
    MaxScode = np.max(-Scode)
    EScode = np.exp(Scode + MaxScode)
    Lcmcm = -np.mean(np.log(np.diagonal(EScode) / (EScode.sum(1) + EPSILON)))
    cmcm_loss = np.float32(0.5 * Lcmcm)

    return (a_q, v_q, a_loss, v_loss,
            a_perplexity, v_perplexity, cmcm_loss, equal_num)
